# revision 8
# baseline (speedup 1.0000x reference)
"""Trainium2 Bass kernel for nn_CSPNet (gnn_message_passing).

Sharding: data-parallel over graphs. G=256 fully-connected graphs of
NPG=24 atoms; edges never cross graphs, so each of the 8 cores owns 32
graphs (768 nodes, 18432 edges) end to end with no collectives.

Device layout is feature-major (transposed): node features live as
[H=128 partitions, nodes], edge tensors as [128, edge-cols] with edge
columns ordered (graph, i=src, j=dst), j fastest. The edge-MLP first
matmul over [h_src, h_dst, lat_ip, dis] (K=325) is decomposed into
  pre1 = A[:,i] + B[:,j] + C[:,g] + D[:,(i,j)] + b1
with A = W1a.T@hT, B = W1b.T@hT (per node), C per graph, and
D = W1d.T@dis (per edge) accumulated in PSUM; A/B/C are applied with
zero-stride broadcast access patterns. The sinusoid edge features are
built once (they do not depend on the layer): per-node sin/cos with
exact fp32 magic-number range reduction, then per-edge angle-addition
products on the vector engine, packed 2 graph-bands deep so the
per-layer D matmul is a single K=60 fp32r matmul per 288-col chunk.
Scatter-mean over src is a grouped free-dim reduce (24 consecutive
j columns -> one node).

fp32r (tf32-like, ~1.5e-4 rel err, full PE rate) is used only for the
two big edge-grain matmuls; everything on the residual stream stays
fp32. The tiny per-graph 3x3 einsums (lat_ip of lattice_feats and the
final lattice_out @ lattices) run on the host: they are ~1e-5 of the
model FLOPs and have no efficient 128-lane mapping.
"""

from contextlib import ExitStack

import numpy as np

import concourse.bacc as bacc
import concourse.mybir as mybir
import concourse.tile as tile
from concourse.bass_utils import run_bass_kernel_spmd

dt = mybir.dt
AF = mybir.ActivationFunctionType
ALU = mybir.AluOpType

# problem shapes (hardcoded per spec)
G, NPG, H, LAT, L, NFREQ, MAXA = 256, 24, 128, 256, 4, 10, 100
N = G * NPG
E = G * NPG * NPG
N_CORES = 8
GPC = G // N_CORES            # 32 graphs per core
NPC = GPC * NPG               # 768 nodes per core
EPC = GPC * NPG * NPG         # 18432 edge cols per core
NBAND = 4                     # bands for the trig product stage
GPB = GPC // NBAND            # 8 graphs per band
NPB = GPB * NPG               # 192 node cols per band
CPB = GPB * NPG * NPG         # 4608 edge cols per band
NB2 = 2                       # bands for dis2 (K=60 contiguous sin+cos)
CPB2 = (GPC // NB2) * NPG * NPG   # 9216 edge cols per dis2 band

MAGIC = float(np.float32(1.5 * 2.0 ** 23))
TWOPI_SAFE = float(2.0 * np.pi * (1.0 - 2.0e-7))

F32, F32R = dt.float32, dt.float32r
import os as _os
# dtype for the edge-grain matmuls (D/A/B): bf16 gets fast-weight-load on
# the PE (4x cheaper per-matmul weight reload); fp32r is ~17x more precise
# but pays a full 512-cycle weight load on every matmul.
EDGE_DT = {"bf16": dt.bfloat16, "f32r": dt.float32r}[
    _os.environ.get("EDGE_DT", "bf16")]

_CACHE = {}


def _build(reps=1):
    """reps>1 wraps the whole kernel in a hardware loop (timing harness)."""
    nc = bacc.Bacc("TRN2", target_bir_lowering=False, debug=False,
                   enable_asserts=True, num_devices=N_CORES)

    def inp(name, shape):
        return nc.dram_tensor(name, list(shape), F32, kind="ExternalInput").ap()

    # per-core data
    I_onehot = inp("onehot", [MAXA, NPC])
    I_xrep = inp("xrep", [NBAND * 3 * NFREQ, NPB])   # [120,192]
    I_freq = inp("freqcol", [NBAND * 3 * NFREQ, 1])
    I_tta = inp("tta", [128, GPC])
    I_ttb = inp("ttb", [128, GPC])
    I_ipt = inp("ipt", [9, GPC])
    # weights (replicated across cores)
    I_emb = inp("emb_table", [MAXA, H])
    I_alw = inp("alw", [H + LAT, H])
    I_alb = inp("alb", [H])
    I_ew1 = inp("ew1", [L, 2 * H + 9 + 6 * NFREQ, H])
    I_eb1 = inp("eb1", [L, H])
    I_ew2 = inp("ew2", [L, H, H])
    I_eb2 = inp("eb2", [L, H])
    I_nw1 = inp("nw1", [L, 2 * H, H])
    I_nb1 = inp("nb1", [L, H])
    I_nw2 = inp("nw2", [L, H, H])
    I_nb2 = inp("nb2", [L, H])
    I_cw = inp("coord_w", [H, 3])
    I_lw = inp("lattice_w", [H, 9])

    O_coord = nc.dram_tensor("coordT", [3, NPC], F32, kind="ExternalOutput").ap()
    O_lat = nc.dram_tensor("latpre", [9, GPC], F32, kind="ExternalOutput").ap()

    R3F = NBAND * 3 * NFREQ     # 120

    with tile.TileContext(nc) as tc:
        with (
            tc.tile_pool(name="const", bufs=1) as pc,
            tc.tile_pool(name="single", bufs=1) as p1,
            tc.tile_pool(name="work", bufs=2) as pw,
            tc.tile_pool(name="trig", bufs=1) as pt,
            tc.tile_pool(name="edge1", bufs=2) as pe1,
            tc.tile_pool(name="edge2", bufs=1) as pe2,
            tc.tile_pool(name="psA", bufs=2, space="PSUM") as psA,
            tc.tile_pool(name="psB", bufs=2, space="PSUM") as psB,
            ExitStack() as _reps_stack,
        ):
            if reps > 1:
                _reps_stack.enter_context(tc.For_i(0, reps, 1))
            # ---------------- load constants ----------------
            onehot = pc.tile([MAXA, NPC], F32)
            nc.sync.dma_start(onehot[:], I_onehot[:])
            emb_t = pc.tile([MAXA, H], F32)
            nc.sync.dma_start(emb_t[:], I_emb[:])
            alw = []
            for k in range(3):
                t = pc.tile([H, H], F32, tag=f"alw{k}")
                nc.sync.dma_start(t[:], I_alw[k * H:(k + 1) * H, :])
                alw.append(t)
            alb = pc.tile([H, 1], F32)
            nc.sync.dma_start(alb[:], I_alb[:].unsqueeze(1))
            tta = pc.tile([128, GPC], F32)
            ttb = pc.tile([128, GPC], F32)
            nc.sync.dma_start(tta[:], I_tta[:])
            nc.sync.dma_start(ttb[:], I_ttb[:])
            ipt = pc.tile([9, GPC], F32)
            nc.sync.dma_start(ipt[:], I_ipt[:])
            cw = pc.tile([H, 3], F32)
            nc.sync.dma_start(cw[:], I_cw[:])
            lw = pc.tile([H, 9], F32)
            nc.sync.dma_start(lw[:], I_lw[:])

            w1a, w1b, w1c, wdr, w2r = [], [], [], [], []
            eb1, eb2, nw1a, nw1b, nb1, nw2, nb2 = [], [], [], [], [], [], []
            for l in range(L):
                wstage = pw.tile([H, H], F32, tag="w1_stage")
                nc.sync.dma_start(wstage[:], I_ew1[l, 0:H, :])
                t = pc.tile([H, H], EDGE_DT, tag=f"w1a{l}")
                nc.vector.tensor_copy(t[:], wstage[:])
                w1a.append(t)
                wstage = pw.tile([H, H], F32, tag="w1_stage")
                nc.sync.dma_start(wstage[:], I_ew1[l, H:2 * H, :])
                t = pc.tile([H, H], EDGE_DT, tag=f"w1b{l}")
                nc.vector.tensor_copy(t[:], wstage[:])
                w1b.append(t)
                t = pc.tile([9, H], F32, tag=f"w1c{l}")
                nc.sync.dma_start(t[:], I_ew1[l, 2 * H:2 * H + 9, :])
                w1c.append(t)
                # W1d duplicated at rows 0:60 and 64:124 so lhsT base
                # partition can match either dis2 band (0 or 64)
                wd = pw.tile([128, H], F32, tag="wd_stage")
                nc.sync.dma_start(wd[0:60, :], I_ew1[l, 2 * H + 9:, :])
                nc.sync.dma_start(wd[64:124, :], I_ew1[l, 2 * H + 9:, :])
                wdr_t = pc.tile([128, H], EDGE_DT, tag=f"wdr{l}")
                nc.vector.tensor_copy(wdr_t[0:60, :], wd[0:60, :])
                nc.vector.tensor_copy(wdr_t[64:124, :], wd[64:124, :])
                wdr.append(wdr_t)
                w2 = pw.tile([H, H], F32, tag="w2_stage")
                nc.sync.dma_start(w2[:], I_ew2[l])
                w2r_t = pc.tile([H, H], EDGE_DT, tag=f"w2r{l}")
                nc.vector.tensor_copy(w2r_t[:], w2[:])
                w2r.append(w2r_t)
                for lst, src, tag in ((eb1, I_eb1, "eb1"), (eb2, I_eb2, "eb2"),
                                      (nb1, I_nb1, "nb1"), (nb2, I_nb2, "nb2")):
                    t = pc.tile([H, 1], F32, tag=f"{tag}{l}")
                    nc.sync.dma_start(t[:], src[l].unsqueeze(1))
                    lst.append(t)
                t = pc.tile([H, H], F32, tag=f"nw1a{l}")
                nc.sync.dma_start(t[:], I_nw1[l, 0:H, :])
                nw1a.append(t)
                t = pc.tile([H, H], F32, tag=f"nw1b{l}")
                nc.sync.dma_start(t[:], I_nw1[l, H:2 * H, :])
                nw1b.append(t)
                t = pc.tile([H, H], F32, tag=f"nw2_{l}")
                nc.sync.dma_start(t[:], I_nw2[l])
                nw2.append(t)

            # ---------------- per-node sin/cos (once) ----------------
            xrep = p1.tile([R3F, NPB], F32)
            nc.sync.dma_start(xrep[:], I_xrep[:])
            freqc = p1.tile([R3F, 1], F32)
            nc.sync.dma_start(freqc[:], I_freq[:])

            v = p1.tile([R3F, NPB], F32, tag="trig_v")
            nc.vector.tensor_scalar_mul(v[:], xrep[:], freqc[:])  # v = f*x
            s4 = pc.tile([R3F, NPB], F32)
            c4 = pc.tile([R3F, NPB], F32)
            for out_t, offs in ((s4, 0.0), (c4, 0.25)):
                vv = pw.tile([R3F, NPB], F32, tag="trig_vv")
                if offs:
                    nc.vector.tensor_scalar_add(vv[:], v[:], offs)
                else:
                    nc.vector.tensor_copy(vv[:], v[:])
                r = pw.tile([R3F, NPB], F32, tag="trig_r")
                nc.vector.tensor_scalar_add(r[:], vv[:], MAGIC)
                nc.vector.tensor_scalar_add(r[:], r[:], -MAGIC)
                u = pw.tile([R3F, NPB], F32, tag="trig_u")
                nc.vector.tensor_tensor(u[:], vv[:], r[:], ALU.subtract)
                nc.vector.tensor_scalar_mul(u[:], u[:], TWOPI_SAFE)
                nc.scalar.activation(out_t[:], u[:], AF.Sin)

            # ---------------- edge sinusoids via angle addition -----------
            # chunk = one graph-pair across all 4 bands: [120, 1152]
            dis2 = pc.tile([128, CPB2], EDGE_DT)  # band b2 at rows 64*b2 (60 used + 4 pad)
            CH = 2 * NPG * NPG          # 1152
            for gp in range(GPB // 2):
                csl = slice(gp * 2 * NPG, (gp + 2) * NPG + gp * NPG)
                csl = slice(gp * 48, gp * 48 + 48)

                def jb(t):
                    a = t[:, csl].rearrange("p (g n) -> p g n", g=2)
                    return a.unsqueeze(2).broadcast_to([R3F, 2, NPG, NPG])

                def ib(t):
                    a = t[:, csl].rearrange("p (g n) -> p g n", g=2)
                    return a.unsqueeze(3).broadcast_to([R3F, 2, NPG, NPG])

                mv = lambda t: t[:].rearrange("p (g i j) -> p g i j", g=2, i=NPG)
                m1 = pt.tile([R3F, CH], F32, tag="m1")
                m2 = pt.tile([R3F, CH], F32, tag="m2")
                nc.vector.tensor_tensor(mv(m1), jb(s4), ib(c4), ALU.mult)
                nc.vector.tensor_tensor(mv(m2), jb(c4), ib(s4), ALU.mult)
                dsin = pt.tile([R3F, CH], EDGE_DT, tag="dsin")
                nc.vector.tensor_tensor(dsin[:], m1[:], m2[:], ALU.subtract)
                m3 = pt.tile([R3F, CH], F32, tag="m1")
                m4 = pt.tile([R3F, CH], F32, tag="m2")
                nc.vector.tensor_tensor(mv(m3), jb(c4), ib(c4), ALU.mult)
                nc.vector.tensor_tensor(mv(m4), jb(s4), ib(s4), ALU.mult)
                dcos = pt.tile([R3F, CH], EDGE_DT, tag="dcos")
                nc.vector.tensor_tensor(dcos[:], m3[:], m4[:], ALU.add)
                # repack into dis2
                for b4 in range(NBAND):
                    b2, hf = b4 // 2, b4 % 2
                    c0 = hf * CPB + gp * CH
                    nc.sync.dma_start(
                        dis2[64 * b2:64 * b2 + 30, c0:c0 + CH],
                        dsin[30 * b4:30 * b4 + 30, :])
                    nc.sync.dma_start(
                        dis2[64 * b2 + 30:64 * b2 + 60, c0:c0 + CH],
                        dcos[30 * b4:30 * b4 + 30, :])

            # ---------------- C for all layers [128, L*GPC] ----------------
            csb = pc.tile([H, L * GPC], F32)
            pC = psA.tile([128, 1024], F32, tag="psA")
            for l in range(L):
                nc.tensor.matmul(pC[:, l * GPC:(l + 1) * GPC], w1c[l][:], ipt[:],
                                 start=True, stop=True)
            nc.scalar.activation(csb[:], pC[:, 0:L * GPC], AF.Copy)

            def two_chunk(ps):   # [128, 2, 384] view of a [128,1024] psum tile
                return ps[:].rearrange("p (k x) -> p k x", k=2)[:, :, 0:384]

            def v2(t):           # [128, 2, 384] view of a [128,768] sbuf tile
                return t[:].rearrange("p (k x) -> p k x", k=2)

            # ---------------- node embedding + time conditioning ----------
            pE = psA.tile([128, 1024], F32, tag="psA")
            for k in range(2):
                nc.tensor.matmul(pE[:, k * 512:k * 512 + 384], emb_t[:],
                                 onehot[:, k * 384:(k + 1) * 384],
                                 start=True, stop=True)
            hemb = p1.tile([H, NPC], F32, tag="hemb")
            nc.scalar.activation(v2(hemb), two_chunk(pE), AF.Copy)

            pH = psB.tile([128, 1024], F32, tag="psB")
            for k in range(2):
                sl = pH[:, k * 512:k * 512 + 384]
                nc.tensor.matmul(sl, alw[0][:], hemb[:, k * 384:(k + 1) * 384],
                                 start=True, stop=False)
                gsl = slice(k * 16, (k + 1) * 16)
                rhs_a = tta[:, gsl].unsqueeze(2).broadcast_to([128, 16, NPG])
                nc.tensor.matmul(sl.rearrange("p (g n) -> p g n", g=16),
                                 alw[1][:], rhs_a, start=False, stop=False)
                rhs_b = ttb[:, gsl].unsqueeze(2).broadcast_to([128, 16, NPG])
                nc.tensor.matmul(sl.rearrange("p (g n) -> p g n", g=16),
                                 alw[2][:], rhs_b, start=False, stop=True)
            hT = pw.tile([H, NPC], F32, tag="hT")
            nc.scalar.activation(v2(hT), two_chunk(pH), AF.Identity, bias=alb[:])

            # ---------------- message-passing layers ----------------
            for l in range(L):
                # fp32r view of h for the edge-grain broadcast matmuls
                hTr = pw.tile([H, NPC], EDGE_DT, tag="hTr")
                nc.vector.tensor_copy(hTr[:], hT[:])
                # silu1 bias per graph: C[:,g] + edge_b1
                csbb = pw.tile([H, GPC], F32, tag="csbb")
                nc.vector.tensor_tensor(
                    csbb[:], csb[:, l * GPC:(l + 1) * GPC],
                    eb1[l][:].broadcast_to([H, GPC]), ALU.add)

                aggsum = p1.tile([H, NPC], F32, tag="aggsum")
                for bb in range(NBAND):       # 8-graph blocks
                    b2, hf = bb // 2, bb % 2
                    ef1 = pe1.tile([H, CPB], EDGE_DT, tag="ef1")
                    for gl in range(GPB):
                        g = bb * GPB + gl
                        col0 = hf * CPB + gl * NPG * NPG
                        pD = psA.tile([128, 1024], F32, tag="psA")
                        # pre1 = D + A_i + B_j accumulated in PSUM:
                        # region hh*512 holds edge cols (i in half hh, j)
                        for hh in range(2):
                            nc.tensor.matmul(
                                pD[:, hh * 512:hh * 512 + 288],
                                wdr[l][64 * b2:64 * b2 + 60, :],
                                dis2[64 * b2:64 * b2 + 60,
                                     col0 + hh * 288:col0 + (hh + 1) * 288],
                                start=True, stop=False)
                        for hh in range(2):
                            rhs_a = (hTr[:, g * NPG + 12 * hh:
                                         g * NPG + 12 * hh + 12]
                                     .unsqueeze(2)
                                     .broadcast_to([H, 12, NPG]))
                            nc.tensor.matmul(
                                pD[:, hh * 512:hh * 512 + 288]
                                .rearrange("p (i j) -> p i j", i=12),
                                w1a[l][:], rhs_a, start=False, stop=False)
                        for hh in range(2):
                            rhs_b = (hTr[:, g * NPG:(g + 1) * NPG]
                                     .unsqueeze(1)
                                     .broadcast_to([H, 12, NPG]))
                            nc.tensor.matmul(
                                pD[:, hh * 512:hh * 512 + 288]
                                .rearrange("p (i j) -> p i j", i=12),
                                w1b[l][:], rhs_b, start=False, stop=True)
                        nc.scalar.activation(
                            ef1[:, gl * 576:(gl + 1) * 576],
                            pD[:].rearrange("p (h x) -> p h x", h=2)[:, :, 0:288],
                            AF.Silu, bias=csbb[:, g:g + 1])
                    # second edge matmul + silu2 over the block
                    ef2 = pe2.tile([H, CPB], F32, tag="ef2")
                    for pp in range(5):          # 9 x 512-col chunks
                        p2 = psB.tile([128, 1024], F32, tag="psB")
                        nchunk = 2 if pp < 4 else 1
                        for cc in range(nchunk):
                            c0 = (pp * 2 + cc) * 512
                            nc.tensor.matmul(p2[:, cc * 512:(cc + 1) * 512],
                                             w2r[l][:], ef1[:, c0:c0 + 512],
                                             start=True, stop=True)
                        nc.scalar.activation(
                            ef2[:, pp * 1024:pp * 1024 + nchunk * 512],
                            p2[:, 0:nchunk * 512], AF.Silu, bias=eb2[l][:])
                    # aggregation: gpsimd pre-halves (24 -> 12 j's), DVE
                    # reduces the remaining 12
                    s1 = pw.tile([H, NPB * 12], F32, tag="aggs1")
                    e_v0 = (ef2[:].rearrange("p (n j2 two) -> p n j2 two",
                                             j2=12, two=2))
                    nc.gpsimd.tensor_tensor(
                        s1[:].rearrange("p (n j2) -> p n j2", j2=12),
                        e_v0[:, :, :, 0], e_v0[:, :, :, 1], ALU.add)
                    nc.vector.tensor_reduce(
                        aggsum[:, bb * NPB:(bb + 1) * NPB],
                        s1[:].rearrange("p (n j2) -> p n j2", j2=12),
                        axis=mybir.AxisListType.X, op=ALU.add)
                aggT = p1.tile([H, NPC], F32, tag="aggT")
                nc.vector.tensor_scalar_mul(aggT[:], aggsum[:], 1.0 / NPG)

                # node MLP + residual
                pN1 = psA.tile([128, 1024], F32, tag="psA")
                for k in range(2):
                    sl = pN1[:, k * 512:k * 512 + 384]
                    nc.tensor.matmul(sl, nw1a[l][:],
                                     hT[:, k * 384:(k + 1) * 384],
                                     start=True, stop=False)
                    nc.tensor.matmul(sl, nw1b[l][:],
                                     aggT[:, k * 384:(k + 1) * 384],
                                     start=False, stop=True)
                upd1 = p1.tile([H, NPC], F32, tag="upd1")
                nc.scalar.activation(v2(upd1), two_chunk(pN1), AF.Silu,
                                     bias=nb1[l][:])
                pN2 = psB.tile([128, 1024], F32, tag="psB")
                for k in range(2):
                    nc.tensor.matmul(pN2[:, k * 512:k * 512 + 384], nw2[l][:],
                                     upd1[:, k * 384:(k + 1) * 384],
                                     start=True, stop=True)
                upd2 = p1.tile([H, NPC], F32, tag="upd2")
                nc.scalar.activation(v2(upd2), two_chunk(pN2), AF.Silu,
                                     bias=nb2[l][:])
                hT_new = pw.tile([H, NPC], F32, tag="hT")
                nc.vector.tensor_tensor(hT_new[:], hT[:], upd2[:], ALU.add)
                hT = hT_new

            # ---------------- outputs ----------------
            pF = psA.tile([128, 1024], F32, tag="psA")
            for k in range(2):
                nc.tensor.matmul(pF[0:3, k * 512:k * 512 + 384], cw[:],
                                 hT[:, k * 384:(k + 1) * 384],
                                 start=True, stop=True)
            coordT = p1.tile([3, NPC], F32, tag="coordT")
            nc.scalar.activation(
                coordT[:].rearrange("p (k x) -> p k x", k=2),
                pF[0:3].rearrange("p (k x) -> p k x", k=2)[:, :, 0:384],
                AF.Copy)
            nc.sync.dma_start(O_coord[:], coordT[:])

            gsum = p1.tile([H, GPC], F32, tag="gsum")
            nc.vector.tensor_reduce(
                gsum[:], hT[:].rearrange("p (g n) -> p g n", g=GPC),
                axis=mybir.AxisListType.X, op=ALU.add)
            gfeat = p1.tile([H, GPC], F32, tag="gfeat")
            nc.vector.tensor_scalar_mul(gfeat[:], gsum[:], 1.0 / NPG)
            pG = psB.tile([128, 1024], F32, tag="psB")
            nc.tensor.matmul(pG[0:9, 0:GPC], lw[:], gfeat[:],
                             start=True, stop=True)
            latp = p1.tile([9, GPC], F32, tag="latp")
            nc.scalar.activation(latp[:], pG[0:9, 0:GPC], AF.Copy)
            nc.sync.dma_start(O_lat[:], latp[:])

    nc.compile()
    return nc


def _prep_inputs(inputs):
    """Host-side sharding + layout prep. Index/layout work only."""
    t = np.asarray(inputs["t"], np.float32)
    frac = np.asarray(inputs["frac_coords"], np.float32)
    lf = np.asarray(inputs["lattice_feats"], np.float32)
    atom_types = np.asarray(inputs["atom_types"])
    node2graph = np.asarray(inputs["node2graph"])
    edge_index = np.asarray(inputs["edge_index"])

    # verify the fully-connected per-graph structure this kernel exploits
    assert node2graph.shape == (N,) and edge_index.shape == (2, E)
    exp_n2g = np.repeat(np.arange(G), NPG)
    assert np.array_equal(node2graph, exp_n2g), "unexpected node2graph"
    offs = (np.arange(G) * NPG)[:, None, None]
    ii = np.arange(NPG)
    exp_src = np.broadcast_to(offs + ii[:, None], (G, NPG, NPG)).reshape(-1)
    exp_dst = np.broadcast_to(offs + ii[None, :], (G, NPG, NPG)).reshape(-1)
    assert np.array_equal(edge_index[0], exp_src), "unexpected edge src"
    assert np.array_equal(edge_index[1], exp_dst), "unexpected edge dst"

    lat_ip = np.einsum("gij,gkj->gik", lf, lf).reshape(G, 9).astype(np.float32)

    freqcol = np.tile(np.arange(NFREQ, dtype=np.float32),
                      NBAND * 3).reshape(-1, 1)

    shared = {
        "emb_table": np.asarray(inputs["emb_table"], np.float32),
        "alw": np.asarray(inputs["atom_latent_w"], np.float32),
        "alb": np.asarray(inputs["atom_latent_b"], np.float32),
        "ew1": np.asarray(inputs["edge_w1"], np.float32),
        "eb1": np.asarray(inputs["edge_b1"], np.float32),
        "ew2": np.asarray(inputs["edge_w2"], np.float32),
        "eb2": np.asarray(inputs["edge_b2"], np.float32),
        "nw1": np.asarray(inputs["node_w1"], np.float32),
        "nb1": np.asarray(inputs["node_b1"], np.float32),
        "nw2": np.asarray(inputs["node_w2"], np.float32),
        "nb2": np.asarray(inputs["node_b2"], np.float32),
        "coord_w": np.asarray(inputs["coord_w"], np.float32),
        "lattice_w": np.asarray(inputs["lattice_w"], np.float32),
        "freqcol": freqcol,
    }

    in_maps = []
    for c in range(N_CORES):
        gsl = slice(c * GPC, (c + 1) * GPC)
        nsl = slice(c * NPC, (c + 1) * NPC)
        at = atom_types[nsl].astype(np.int64) - 1
        onehot = np.zeros((MAXA, NPC), np.float32)
        onehot[at, np.arange(NPC)] = 1.0
        xc = frac[nsl]                                 # [768,3]
        xb = xc.reshape(NBAND, NPB, 3).transpose(0, 2, 1)   # [4,3,192]
        xrep = np.repeat(xb[:, :, None, :], NFREQ, axis=2).reshape(120, NPB)
        tc_ = t[gsl]                                   # [32,256]
        in_maps.append(dict(shared,
                            onehot=onehot,
                            xrep=np.ascontiguousarray(xrep),
                            tta=np.ascontiguousarray(tc_[:, 0:128].T),
                            ttb=np.ascontiguousarray(tc_[:, 128:256].T),
                            ipt=np.ascontiguousarray(lat_ip[gsl].T)))
    return in_maps


def kernel(**inputs):
    if "nc" not in _CACHE:
        _CACHE["nc"] = _build()
    nc = _CACHE["nc"]
    in_maps = _prep_inputs(inputs)
    res = run_bass_kernel_spmd(nc, in_maps, list(range(N_CORES)))
    coord = np.concatenate([res.results[c]["coordT"].T
                            for c in range(N_CORES)], axis=0)
    lo = np.concatenate([res.results[c]["latpre"].T.reshape(GPC, 3, 3)
                         for c in range(N_CORES)], axis=0)
    lattices = np.asarray(inputs["lattices"], np.float32)
    lattice_out = np.einsum("bij,bjk->bik", lo, lattices).astype(np.float32)
    return lattice_out, coord.astype(np.float32)


# revision 10
# speedup vs baseline: 1.0824x; 1.0824x over previous
"""Trainium2 Bass kernel for nn_CSPNet (gnn_message_passing).

Sharding: data-parallel over graphs. G=256 fully-connected graphs of
NPG=24 atoms; edges never cross graphs, so each of the 8 cores owns 32
graphs (768 nodes, 18432 edges) end to end with no collectives.

Device layout is feature-major (transposed): node features live as
[H=128 partitions, nodes], edge tensors as [128, edge-cols] with edge
columns ordered (graph, i=src, j=dst), j fastest. The edge-MLP first
matmul over [h_src, h_dst, lat_ip, dis] (K=325) is decomposed into
  pre1 = A[:,i] + B[:,j] + C[:,g] + D[:,(i,j)] + b1
with A/B per-node matmuls, C per graph, and D = W1d.T@dis per edge
(PSUM); the broadcasts are zero-stride access patterns applied by the
vector engine (A', reads PSUM) and GPSIMD (B, SBUF only). Sinusoid
features are layer-independent and built once: per-node sin/cos with
exact fp32 magic-number range reduction, then per-edge angle-addition
products, packed 2 graph-bands deep (64-partition stride) so the
per-layer D matmul is one K=60 matmul per 288-col chunk. Scatter-mean
over src is a grouped free-dim reduce (24 consecutive j cols -> node).

The 0.02-scaled weights contract edge-stage rounding noise by ~50x per
layer, so the edge-grain tensors and matmuls run in bf16 (measured
~1e-6 end-to-end error) which enables PE fast-weight-load and the DVE
2x reduce mode; the residual stream (h, node MLP, outputs) stays fp32.
The tiny per-graph 3x3 einsums (lat_ip and lattice_out @ lattices) run
on the host: ~1e-5 of FLOPs with no efficient 128-lane mapping.
"""

from contextlib import ExitStack

import numpy as np

import concourse.bacc as bacc
import concourse.mybir as mybir
import concourse.tile as tile
from concourse.bass_utils import run_bass_kernel_spmd

dt = mybir.dt
AF = mybir.ActivationFunctionType
ALU = mybir.AluOpType

# problem shapes (hardcoded per spec)
G, NPG, H, LAT, L, NFREQ, MAXA = 256, 24, 128, 256, 4, 10, 100
N = G * NPG
E = G * NPG * NPG
N_CORES = 8
GPC = G // N_CORES            # 32 graphs per core
NPC = GPC * NPG               # 768 nodes per core
EPC = GPC * NPG * NPG         # 18432 edge cols per core
NBAND = 4                     # bands for the trig product stage
GPB = GPC // NBAND            # 8 graphs per band
NPB = GPB * NPG               # 192 node cols per band
CPB = GPB * NPG * NPG         # 4608 edge cols per band
NB2 = 2                       # bands for dis2 (K=60 contiguous sin+cos)
CPB2 = (GPC // NB2) * NPG * NPG   # 9216 edge cols per dis2 band

MAGIC = float(np.float32(1.5 * 2.0 ** 23))
TWOPI_SAFE = float(2.0 * np.pi * (1.0 - 2.0e-7))

F32, F32R = dt.float32, dt.float32r
import os as _os
EDGE_DT = {"bf16": dt.bfloat16, "f32r": dt.float32r}[
    _os.environ.get("EDGE_DT", "bf16")]

_CACHE = {}


def _build(reps=1):
    """reps>1 wraps the whole kernel in a hardware loop (timing harness)."""
    nc = bacc.Bacc("TRN2", target_bir_lowering=False, debug=False,
                   enable_asserts=True, num_devices=N_CORES)

    def inp(name, shape):
        return nc.dram_tensor(name, list(shape), F32, kind="ExternalInput").ap()

    # per-core data
    I_onehot = inp("onehot", [MAXA, NPC])
    I_xrep = inp("xrep", [NBAND * 3 * NFREQ, NPB])   # [120,192]
    I_freq = inp("freqcol", [NBAND * 3 * NFREQ, 1])
    I_tta = inp("tta", [128, GPC])
    I_ttb = inp("ttb", [128, GPC])
    I_ipt = inp("ipt", [9, GPC])
    # weights (replicated across cores)
    I_emb = inp("emb_table", [MAXA, H])
    I_alw = inp("alw", [H + LAT, H])
    I_alb = inp("alb", [H])
    I_ew1 = inp("ew1", [L, 2 * H + 9 + 6 * NFREQ, H])
    I_eb1 = inp("eb1", [L, H])
    I_ew2 = inp("ew2", [L, H, H])
    I_eb2 = inp("eb2", [L, H])
    I_nw1 = inp("nw1", [L, 2 * H, H])
    I_nb1 = inp("nb1", [L, H])
    I_nw2 = inp("nw2", [L, H, H])
    I_nb2 = inp("nb2", [L, H])
    I_cw = inp("coord_w", [H, 3])
    I_lw = inp("lattice_w", [H, 9])

    O_coord = nc.dram_tensor("coordT", [3, NPC], F32, kind="ExternalOutput").ap()
    O_lat = nc.dram_tensor("latpre", [9, GPC], F32, kind="ExternalOutput").ap()

    R3F = NBAND * 3 * NFREQ     # 120

    with tile.TileContext(nc) as tc:
        with (
            tc.tile_pool(name="const", bufs=1) as pc,
            tc.tile_pool(name="single", bufs=1) as p1,
            tc.tile_pool(name="work", bufs=2) as pw,
            tc.tile_pool(name="trig", bufs=1) as pt,
            tc.tile_pool(name="edge1", bufs=2) as pe1,
            tc.tile_pool(name="edge2", bufs=1) as pe2,
            tc.tile_pool(name="psA", bufs=2, space="PSUM") as psA,
            tc.tile_pool(name="psB", bufs=2, space="PSUM") as psB,
            ExitStack() as _reps_stack,
        ):
            if reps > 1:
                _reps_stack.enter_context(tc.For_i(0, reps, 1))
            # ---------------- load constants ----------------
            onehot = pc.tile([MAXA, NPC], F32)
            nc.sync.dma_start(onehot[:], I_onehot[:])
            emb_t = pc.tile([MAXA, H], F32)
            nc.sync.dma_start(emb_t[:], I_emb[:])
            alw = []
            for k in range(3):
                t = pc.tile([H, H], F32, tag=f"alw{k}")
                nc.sync.dma_start(t[:], I_alw[k * H:(k + 1) * H, :])
                alw.append(t)
            alb = pc.tile([H, 1], F32)
            nc.sync.dma_start(alb[:], I_alb[:].unsqueeze(1))
            tta = pc.tile([128, GPC], F32)
            ttb = pc.tile([128, GPC], F32)
            nc.sync.dma_start(tta[:], I_tta[:])
            nc.sync.dma_start(ttb[:], I_ttb[:])
            ipt = pc.tile([9, GPC], F32)
            nc.sync.dma_start(ipt[:], I_ipt[:])
            cw = pc.tile([H, 3], F32)
            nc.sync.dma_start(cw[:], I_cw[:])
            lw = pc.tile([H, 9], F32)
            nc.sync.dma_start(lw[:], I_lw[:])

            w1a, w1b, w1c, wdr, w2r = [], [], [], [], []
            eb1, eb2, nw1a, nw1b, nb1, nw2, nb2 = [], [], [], [], [], [], []
            for l in range(L):
                t = pc.tile([H, H], F32, tag=f"w1a{l}")
                nc.sync.dma_start(t[:], I_ew1[l, 0:H, :])
                w1a.append(t)
                t = pc.tile([H, H], F32, tag=f"w1b{l}")
                nc.sync.dma_start(t[:], I_ew1[l, H:2 * H, :])
                w1b.append(t)
                t = pc.tile([9, H], F32, tag=f"w1c{l}")
                nc.sync.dma_start(t[:], I_ew1[l, 2 * H:2 * H + 9, :])
                w1c.append(t)
                # W1d duplicated at rows 0:60 and 64:124 so lhsT base
                # partition can match either dis2 band (0 or 64)
                wd = pw.tile([128, H], F32, tag="wd_stage")
                nc.sync.dma_start(wd[0:60, :], I_ew1[l, 2 * H + 9:, :])
                nc.sync.dma_start(wd[64:124, :], I_ew1[l, 2 * H + 9:, :])
                wdr_t = pc.tile([128, H], EDGE_DT, tag=f"wdr{l}")
                nc.vector.tensor_copy(wdr_t[0:60, :], wd[0:60, :])
                nc.vector.tensor_copy(wdr_t[64:124, :], wd[64:124, :])
                wdr.append(wdr_t)
                w2 = pw.tile([H, H], F32, tag="w2_stage")
                nc.sync.dma_start(w2[:], I_ew2[l])
                w2r_t = pc.tile([H, H], EDGE_DT, tag=f"w2r{l}")
                nc.vector.tensor_copy(w2r_t[:], w2[:])
                w2r.append(w2r_t)
                for lst, srcw, tag in ((eb1, I_eb1, "eb1"), (eb2, I_eb2, "eb2"),
                                       (nb1, I_nb1, "nb1"), (nb2, I_nb2, "nb2")):
                    t = pc.tile([H, 1], F32, tag=f"{tag}{l}")
                    nc.sync.dma_start(t[:], srcw[l].unsqueeze(1))
                    lst.append(t)
                t = pc.tile([H, H], F32, tag=f"nw1a{l}")
                nc.sync.dma_start(t[:], I_nw1[l, 0:H, :])
                nw1a.append(t)
                t = pc.tile([H, H], F32, tag=f"nw1b{l}")
                nc.sync.dma_start(t[:], I_nw1[l, H:2 * H, :])
                nw1b.append(t)
                t = pc.tile([H, H], F32, tag=f"nw2_{l}")
                nc.sync.dma_start(t[:], I_nw2[l])
                nw2.append(t)

            # ---------------- per-node sin/cos (once) ----------------
            xrep = p1.tile([R3F, NPB], F32)
            nc.sync.dma_start(xrep[:], I_xrep[:])
            freqc = p1.tile([R3F, 1], F32)
            nc.sync.dma_start(freqc[:], I_freq[:])

            v = p1.tile([R3F, NPB], F32, tag="trig_v")
            nc.vector.tensor_scalar_mul(v[:], xrep[:], freqc[:])  # v = f*x
            s4 = pc.tile([R3F, NPB], F32)
            c4 = pc.tile([R3F, NPB], F32)
            for out_t, offs in ((s4, 0.0), (c4, 0.25)):
                vv = pw.tile([R3F, NPB], F32, tag="trig_vv")
                if offs:
                    nc.vector.tensor_scalar_add(vv[:], v[:], offs)
                else:
                    nc.vector.tensor_copy(vv[:], v[:])
                r = pw.tile([R3F, NPB], F32, tag="trig_r")
                nc.vector.tensor_scalar_add(r[:], vv[:], MAGIC)
                nc.vector.tensor_scalar_add(r[:], r[:], -MAGIC)
                u = pw.tile([R3F, NPB], F32, tag="trig_u")
                nc.vector.tensor_tensor(u[:], vv[:], r[:], ALU.subtract)
                nc.vector.tensor_scalar_mul(u[:], u[:], TWOPI_SAFE)
                nc.scalar.activation(out_t[:], u[:], AF.Sin)

            # ---------------- edge sinusoids via angle addition -----------
            # chunk = one graph-pair across all 4 bands: [120, 1152]
            dis2 = pc.tile([128, CPB2], EDGE_DT)  # band b2 at rows 64*b2
            CH = 2 * NPG * NPG          # 1152
            for gp in range(GPB // 2):
                csl = slice(gp * 48, gp * 48 + 48)

                def jb(t):
                    a = t[:, csl].rearrange("p (g n) -> p g n", g=2)
                    return a.unsqueeze(2).broadcast_to([R3F, 2, NPG, NPG])

                def ib(t):
                    a = t[:, csl].rearrange("p (g n) -> p g n", g=2)
                    return a.unsqueeze(3).broadcast_to([R3F, 2, NPG, NPG])

                mv = lambda t: t[:].rearrange("p (g i j) -> p g i j", g=2, i=NPG)
                m1 = pt.tile([R3F, CH], F32, tag="m1")
                m2 = pt.tile([R3F, CH], F32, tag="m2")
                nc.vector.tensor_tensor(mv(m1), jb(s4), ib(c4), ALU.mult)
                nc.vector.tensor_tensor(mv(m2), jb(c4), ib(s4), ALU.mult)
                dsin = pt.tile([R3F, CH], EDGE_DT, tag="dsin")
                nc.vector.tensor_tensor(dsin[:], m1[:], m2[:], ALU.subtract)
                m3 = pt.tile([R3F, CH], F32, tag="m1")
                m4 = pt.tile([R3F, CH], F32, tag="m2")
                nc.vector.tensor_tensor(mv(m3), jb(c4), ib(c4), ALU.mult)
                nc.vector.tensor_tensor(mv(m4), jb(s4), ib(s4), ALU.mult)
                dcos = pt.tile([R3F, CH], EDGE_DT, tag="dcos")
                nc.vector.tensor_tensor(dcos[:], m3[:], m4[:], ALU.add)
                # repack into dis2
                for b4 in range(NBAND):
                    b2, hf = b4 // 2, b4 % 2
                    c0 = hf * CPB + gp * CH
                    nc.sync.dma_start(
                        dis2[64 * b2:64 * b2 + 30, c0:c0 + CH],
                        dsin[30 * b4:30 * b4 + 30, :])
                    nc.sync.dma_start(
                        dis2[64 * b2 + 30:64 * b2 + 60, c0:c0 + CH],
                        dcos[30 * b4:30 * b4 + 30, :])

            # ---------------- C for all layers [128, L*GPC] ----------------
            csb = pc.tile([H, L * GPC], F32)
            pC = psA.tile([128, 1024], F32, tag="psA")
            for l in range(L):
                nc.tensor.matmul(pC[:, l * GPC:(l + 1) * GPC], w1c[l][:], ipt[:],
                                 start=True, stop=True)
            nc.scalar.activation(csb[:], pC[:, 0:L * GPC], AF.Copy)

            def two_chunk(ps):   # [128, 2, 384] view of a [128,1024] psum tile
                return ps[:].rearrange("p (k x) -> p k x", k=2)[:, :, 0:384]

            def v2(t):           # [128, 2, 384] view of a [128,768] sbuf tile
                return t[:].rearrange("p (k x) -> p k x", k=2)

            # ---------------- node embedding + time conditioning ----------
            pE = psA.tile([128, 1024], F32, tag="psA")
            for k in range(2):
                nc.tensor.matmul(pE[:, k * 512:k * 512 + 384], emb_t[:],
                                 onehot[:, k * 384:(k + 1) * 384],
                                 start=True, stop=True)
            hemb = p1.tile([H, NPC], F32, tag="hemb")
            nc.scalar.activation(v2(hemb), two_chunk(pE), AF.Copy)

            pH = psB.tile([128, 1024], F32, tag="psB")
            for k in range(2):
                sl = pH[:, k * 512:k * 512 + 384]
                nc.tensor.matmul(sl, alw[0][:], hemb[:, k * 384:(k + 1) * 384],
                                 start=True, stop=False)
                gsl = slice(k * 16, (k + 1) * 16)
                rhs_a = tta[:, gsl].unsqueeze(2).broadcast_to([128, 16, NPG])
                nc.tensor.matmul(sl.rearrange("p (g n) -> p g n", g=16),
                                 alw[1][:], rhs_a, start=False, stop=False)
                rhs_b = ttb[:, gsl].unsqueeze(2).broadcast_to([128, 16, NPG])
                nc.tensor.matmul(sl.rearrange("p (g n) -> p g n", g=16),
                                 alw[2][:], rhs_b, start=False, stop=True)
            hT = pw.tile([H, NPC], F32, tag="hT")
            nc.scalar.activation(v2(hT), two_chunk(pH), AF.Identity, bias=alb[:])

            # ---------------- message-passing layers ----------------
            for l in range(L):
                # A' = W1a.T @ hT + C bcast ;  B = W1b.T @ hT   (fp32 mms)
                pA = psA.tile([128, 1024], F32, tag="psA")
                pB = psB.tile([128, 1024], F32, tag="psB")
                for k in range(2):
                    nc.tensor.matmul(pA[:, k * 512:k * 512 + 384], w1a[l][:],
                                     hT[:, k * 384:(k + 1) * 384],
                                     start=True, stop=True)
                    nc.tensor.matmul(pB[:, k * 512:k * 512 + 384], w1b[l][:],
                                     hT[:, k * 384:(k + 1) * 384],
                                     start=True, stop=True)
                aP = p1.tile([H, NPC], F32, tag="aP")
                cb = csb[:, l * GPC:(l + 1) * GPC]
                cb4 = (cb.rearrange("p (k g) -> p k g", k=2)
                       .unsqueeze(3).broadcast_to([H, 2, 16, NPG]))
                in0 = (pA[:].rearrange("p (k x) -> p k x", k=2)[:, :, 0:384]
                       .rearrange("p k (g n) -> p k g n", n=NPG))
                out0 = (aP[:].rearrange("p (k g n) -> p k g n", k=2, g=16))
                nc.vector.tensor_tensor(out0, in0, cb4, ALU.add)
                bP = p1.tile([H, NPC], EDGE_DT, tag="bP")
                nc.scalar.activation(
                    bP[:].rearrange("p (k x) -> p k x", k=2),
                    pB[:].rearrange("p (k x) -> p k x", k=2)[:, :, 0:384],
                    AF.Copy)

                aggsum = p1.tile([H, NPC], F32, tag="aggsum")
                for bb in range(NBAND):       # 8-graph blocks
                    b2, hf = bb // 2, bb % 2
                    ef1 = pe1.tile([H, CPB], EDGE_DT, tag="ef1")
                    for gp in range(GPB // 2):   # graph pairs
                        tmp2 = pw.tile([H, 2 * NPG * NPG], F32, tag="tmp2")
                        for gh in range(2):
                            gl = gp * 2 + gh
                            g = bb * GPB + gl
                            col0 = hf * CPB + gl * NPG * NPG
                            pD = psA.tile([128, 1024], F32, tag="psA")
                            for hh in range(2):
                                nc.tensor.matmul(
                                    pD[:, hh * 512:hh * 512 + 288],
                                    wdr[l][64 * b2:64 * b2 + 60, :],
                                    dis2[64 * b2:64 * b2 + 60,
                                         col0 + hh * 288:col0 + (hh + 1) * 288],
                                    start=True, stop=True)
                            # pass1: tmp2 slice = psumD + A' bcast   (DVE)
                            a_sl = aP[:, g * NPG:(g + 1) * NPG]
                            a_v = (a_sl.rearrange("p (h i) -> p h i", h=2)
                                   .unsqueeze(3).broadcast_to([H, 2, 12, NPG]))
                            o_v = (tmp2[:, gh * 576:(gh + 1) * 576]
                                   .rearrange("p (h i j) -> p h i j", h=2, i=12))
                            i_v = (pD[:].rearrange("p (h x) -> p h x", h=2)
                                   [:, :, 0:288]
                                   .rearrange("p h (i j) -> p h i j", i=12))
                            nc.vector.tensor_tensor(o_v, i_v, a_v, ALU.add)
                        # pass2 on gpsimd: pre1 = tmp2 + B bcast
                        g0 = bb * GPB + gp * 2
                        b_sl = bP[:, g0 * NPG:(g0 + 2) * NPG]
                        b_v = (b_sl.rearrange("p (g j) -> p g j", g=2)
                               .unsqueeze(2).broadcast_to([H, 2, NPG, NPG]))
                        pre1 = pw.tile([H, 2 * NPG * NPG], F32, tag="pre1")
                        pv = lambda t: t[:].rearrange("p (g i j) -> p g i j",
                                                      g=2, i=NPG)
                        nc.gpsimd.tensor_tensor(pv(pre1), pv(tmp2), b_v, ALU.add)
                        nc.scalar.activation(
                            ef1[:, gp * 1152:(gp + 1) * 1152], pre1[:],
                            AF.Silu, bias=eb1[l][:])
                    # second edge matmul + silu2 over the block
                    ef2 = pe2.tile([H, CPB], EDGE_DT, tag="ef2")
                    for pp in range(5):          # 9 x 512-col chunks
                        p2 = psB.tile([128, 1024], F32, tag="psB")
                        nchunk = 2 if pp < 4 else 1
                        for cc in range(nchunk):
                            c0 = (pp * 2 + cc) * 512
                            nc.tensor.matmul(p2[:, cc * 512:(cc + 1) * 512],
                                             w2r[l][:], ef1[:, c0:c0 + 512],
                                             start=True, stop=True)
                        nc.scalar.activation(
                            ef2[:, pp * 1024:pp * 1024 + nchunk * 512],
                            p2[:, 0:nchunk * 512], AF.Silu, bias=eb2[l][:])
                    # aggregation over j; bf16 input gets the DVE 2x mode
                    nc.vector.tensor_reduce(
                        aggsum[:, bb * NPB:(bb + 1) * NPB],
                        ef2[:].rearrange("p (n j) -> p n j", j=NPG),
                        axis=mybir.AxisListType.X, op=ALU.add)
                aggT = p1.tile([H, NPC], F32, tag="aggT")
                nc.vector.tensor_scalar_mul(aggT[:], aggsum[:], 1.0 / NPG)

                # node MLP + residual
                pN1 = psA.tile([128, 1024], F32, tag="psA")
                for k in range(2):
                    sl = pN1[:, k * 512:k * 512 + 384]
                    nc.tensor.matmul(sl, nw1a[l][:],
                                     hT[:, k * 384:(k + 1) * 384],
                                     start=True, stop=False)
                    nc.tensor.matmul(sl, nw1b[l][:],
                                     aggT[:, k * 384:(k + 1) * 384],
                                     start=False, stop=True)
                upd1 = p1.tile([H, NPC], F32, tag="upd1")
                nc.scalar.activation(v2(upd1), two_chunk(pN1), AF.Silu,
                                     bias=nb1[l][:])
                pN2 = psB.tile([128, 1024], F32, tag="psB")
                for k in range(2):
                    nc.tensor.matmul(pN2[:, k * 512:k * 512 + 384], nw2[l][:],
                                     upd1[:, k * 384:(k + 1) * 384],
                                     start=True, stop=True)
                upd2 = p1.tile([H, NPC], F32, tag="upd2")
                nc.scalar.activation(v2(upd2), two_chunk(pN2), AF.Silu,
                                     bias=nb2[l][:])
                hT_new = pw.tile([H, NPC], F32, tag="hT")
                nc.vector.tensor_tensor(hT_new[:], hT[:], upd2[:], ALU.add)
                hT = hT_new

            # ---------------- outputs ----------------
            pF = psA.tile([128, 1024], F32, tag="psA")
            for k in range(2):
                nc.tensor.matmul(pF[0:3, k * 512:k * 512 + 384], cw[:],
                                 hT[:, k * 384:(k + 1) * 384],
                                 start=True, stop=True)
            coordT = p1.tile([3, NPC], F32, tag="coordT")
            nc.scalar.activation(
                coordT[:].rearrange("p (k x) -> p k x", k=2),
                pF[0:3].rearrange("p (k x) -> p k x", k=2)[:, :, 0:384],
                AF.Copy)
            nc.sync.dma_start(O_coord[:], coordT[:])

            gsum = p1.tile([H, GPC], F32, tag="gsum")
            nc.vector.tensor_reduce(
                gsum[:], hT[:].rearrange("p (g n) -> p g n", g=GPC),
                axis=mybir.AxisListType.X, op=ALU.add)
            gfeat = p1.tile([H, GPC], F32, tag="gfeat")
            nc.vector.tensor_scalar_mul(gfeat[:], gsum[:], 1.0 / NPG)
            pG = psB.tile([128, 1024], F32, tag="psB")
            nc.tensor.matmul(pG[0:9, 0:GPC], lw[:], gfeat[:],
                             start=True, stop=True)
            latp = p1.tile([9, GPC], F32, tag="latp")
            nc.scalar.activation(latp[:], pG[0:9, 0:GPC], AF.Copy)
            nc.sync.dma_start(O_lat[:], latp[:])

    nc.compile()
    return nc


def _prep_inputs(inputs):
    """Host-side sharding + layout prep. Index/layout work only."""
    t = np.asarray(inputs["t"], np.float32)
    frac = np.asarray(inputs["frac_coords"], np.float32)
    lf = np.asarray(inputs["lattice_feats"], np.float32)
    atom_types = np.asarray(inputs["atom_types"])
    node2graph = np.asarray(inputs["node2graph"])
    edge_index = np.asarray(inputs["edge_index"])

    # verify the fully-connected per-graph structure this kernel exploits
    assert node2graph.shape == (N,) and edge_index.shape == (2, E)
    exp_n2g = np.repeat(np.arange(G), NPG)
    assert np.array_equal(node2graph, exp_n2g), "unexpected node2graph"
    offs = (np.arange(G) * NPG)[:, None, None]
    ii = np.arange(NPG)
    exp_src = np.broadcast_to(offs + ii[:, None], (G, NPG, NPG)).reshape(-1)
    exp_dst = np.broadcast_to(offs + ii[None, :], (G, NPG, NPG)).reshape(-1)
    assert np.array_equal(edge_index[0], exp_src), "unexpected edge src"
    assert np.array_equal(edge_index[1], exp_dst), "unexpected edge dst"

    lat_ip = np.einsum("gij,gkj->gik", lf, lf).reshape(G, 9).astype(np.float32)

    freqcol = np.tile(np.arange(NFREQ, dtype=np.float32),
                      NBAND * 3).reshape(-1, 1)

    shared = {
        "emb_table": np.asarray(inputs["emb_table"], np.float32),
        "alw": np.asarray(inputs["atom_latent_w"], np.float32),
        "alb": np.asarray(inputs["atom_latent_b"], np.float32),
        "ew1": np.asarray(inputs["edge_w1"], np.float32),
        "eb1": np.asarray(inputs["edge_b1"], np.float32),
        "ew2": np.asarray(inputs["edge_w2"], np.float32),
        "eb2": np.asarray(inputs["edge_b2"], np.float32),
        "nw1": np.asarray(inputs["node_w1"], np.float32),
        "nb1": np.asarray(inputs["node_b1"], np.float32),
        "nw2": np.asarray(inputs["node_w2"], np.float32),
        "nb2": np.asarray(inputs["node_b2"], np.float32),
        "coord_w": np.asarray(inputs["coord_w"], np.float32),
        "lattice_w": np.asarray(inputs["lattice_w"], np.float32),
        "freqcol": freqcol,
    }

    in_maps = []
    for c in range(N_CORES):
        gsl = slice(c * GPC, (c + 1) * GPC)
        nsl = slice(c * NPC, (c + 1) * NPC)
        at = atom_types[nsl].astype(np.int64) - 1
        onehot = np.zeros((MAXA, NPC), np.float32)
        onehot[at, np.arange(NPC)] = 1.0
        xc = frac[nsl]                                 # [768,3]
        xb = xc.reshape(NBAND, NPB, 3).transpose(0, 2, 1)   # [4,3,192]
        xrep = np.repeat(xb[:, :, None, :], NFREQ, axis=2).reshape(120, NPB)
        tc_ = t[gsl]                                   # [32,256]
        in_maps.append(dict(shared,
                            onehot=onehot,
                            xrep=np.ascontiguousarray(xrep),
                            tta=np.ascontiguousarray(tc_[:, 0:128].T),
                            ttb=np.ascontiguousarray(tc_[:, 128:256].T),
                            ipt=np.ascontiguousarray(lat_ip[gsl].T)))
    return in_maps


def kernel(**inputs):
    if "nc" not in _CACHE:
        _CACHE["nc"] = _build()
    nc = _CACHE["nc"]
    in_maps = _prep_inputs(inputs)
    res = run_bass_kernel_spmd(nc, in_maps, list(range(N_CORES)))
    coord = np.concatenate([res.results[c]["coordT"].T
                            for c in range(N_CORES)], axis=0)
    lo = np.concatenate([res.results[c]["latpre"].T.reshape(GPC, 3, 3)
                         for c in range(N_CORES)], axis=0)
    lattices = np.asarray(inputs["lattices"], np.float32)
    lattice_out = np.einsum("bij,bjk->bik", lo, lattices).astype(np.float32)
    return lattice_out, coord.astype(np.float32)


# revision 11
# speedup vs baseline: 1.4515x; 1.3411x over previous
"""Trainium2 Bass kernel for nn_CSPNet (gnn_message_passing).

Sharding: data-parallel over graphs. G=256 fully-connected graphs of
NPG=24 atoms; edges never cross graphs, so each of the 8 cores owns 32
graphs (768 nodes, 18432 edges) end to end with no collectives.

Device layout is feature-major (transposed): node features live as
[H=128 partitions, nodes], edge tensors as [128, edge-cols] with edge
columns ordered (graph, i=src, j=dst), j fastest. The edge-MLP first
matmul over [h_src, h_dst, lat_ip, dis] (K=325) is decomposed into
  pre1 = A[:,i] + B[:,j] + C[:,g] + D[:,(i,j)] + b1
with A/B per-node matmuls, C per graph, and D = W1d.T@dis per edge
(PSUM); the broadcasts are zero-stride access patterns applied by the
vector engine (A', reads PSUM) and GPSIMD (B, SBUF only). Sinusoid
features are layer-independent and built once: per-node sin/cos with
exact fp32 magic-number range reduction, then per-edge angle-addition
products, packed 2 graph-bands deep (64-partition stride) so the
per-layer D matmul is one K=60 matmul per 288-col chunk. Scatter-mean
over src is a grouped free-dim reduce (24 consecutive j cols -> node).

The 0.02-scaled weights contract edge-stage rounding noise by ~50x per
layer, so the edge-grain tensors and matmuls run in bf16 (measured
~1e-6 end-to-end error) which enables PE fast-weight-load and the DVE
2x reduce mode; the residual stream (h, node MLP, outputs) stays fp32.
The tiny per-graph 3x3 einsums (lat_ip and lattice_out @ lattices) run
on the host: ~1e-5 of FLOPs with no efficient 128-lane mapping.
"""

from contextlib import ExitStack

import numpy as np

import concourse.bacc as bacc
import concourse.mybir as mybir
import concourse.tile as tile
from concourse.bass_utils import run_bass_kernel_spmd

dt = mybir.dt
AF = mybir.ActivationFunctionType
ALU = mybir.AluOpType

# problem shapes (hardcoded per spec)
G, NPG, H, LAT, L, NFREQ, MAXA = 256, 24, 128, 256, 4, 10, 100
N = G * NPG
E = G * NPG * NPG
N_CORES = 8
GPC = G // N_CORES            # 32 graphs per core
NPC = GPC * NPG               # 768 nodes per core
EPC = GPC * NPG * NPG         # 18432 edge cols per core
NBAND = 4                     # bands for the trig product stage
GPB = GPC // NBAND            # 8 graphs per band
NPB = GPB * NPG               # 192 node cols per band
CPB = GPB * NPG * NPG         # 4608 edge cols per band
NB2 = 2                       # bands for dis2 (K=60 contiguous sin+cos)
CPB2 = (GPC // NB2) * NPG * NPG   # 9216 edge cols per dis2 band

MAGIC = float(np.float32(1.5 * 2.0 ** 23))
TWOPI_SAFE = float(2.0 * np.pi * (1.0 - 2.0e-7))

F32, F32R = dt.float32, dt.float32r
import os as _os
EDGE_DT = {"bf16": dt.bfloat16, "f32r": dt.float32r}[
    _os.environ.get("EDGE_DT", "bf16")]

_CACHE = {}


def _build(reps=1):
    """reps>1 wraps the whole kernel in a hardware loop (timing harness)."""
    nc = bacc.Bacc("TRN2", target_bir_lowering=False, debug=False,
                   enable_asserts=True, num_devices=N_CORES)

    def inp(name, shape):
        return nc.dram_tensor(name, list(shape), F32, kind="ExternalInput").ap()

    # per-core data
    I_onehot = inp("onehot", [MAXA, NPC])
    I_xrep = inp("xrep", [NBAND * 3 * NFREQ, NPB])   # [120,192]
    I_freq = inp("freqcol", [NBAND * 3 * NFREQ, 1])
    I_tta = inp("tta", [128, GPC])
    I_ttb = inp("ttb", [128, GPC])
    I_ipt = inp("ipt", [9, GPC])
    # weights (replicated across cores)
    I_emb = inp("emb_table", [MAXA, H])
    I_alw = inp("alw", [H + LAT, H])
    I_alb = inp("alb", [H])
    I_ew1 = inp("ew1", [L, 2 * H + 9 + 6 * NFREQ, H])
    I_eb1 = inp("eb1", [L, H])
    I_ew2 = inp("ew2", [L, H, H])
    I_eb2 = inp("eb2", [L, H])
    I_nw1 = inp("nw1", [L, 2 * H, H])
    I_nb1 = inp("nb1", [L, H])
    I_nw2 = inp("nw2", [L, H, H])
    I_nb2 = inp("nb2", [L, H])
    I_cw = inp("coord_w", [H, 3])
    I_lw = inp("lattice_w", [H, 9])

    O_coord = nc.dram_tensor("coordT", [3, NPC], F32, kind="ExternalOutput").ap()
    O_lat = nc.dram_tensor("latpre", [9, GPC], F32, kind="ExternalOutput").ap()

    R3F = NBAND * 3 * NFREQ     # 120

    with tile.TileContext(nc) as tc:
        with (
            tc.tile_pool(name="const", bufs=1) as pc,
            tc.tile_pool(name="single", bufs=1) as p1,
            tc.tile_pool(name="work", bufs=2) as pw,
            tc.tile_pool(name="trig", bufs=1) as pt,
            tc.tile_pool(name="edge1", bufs=2) as pe1,
            tc.tile_pool(name="edge2", bufs=1) as pe2,
            tc.tile_pool(name="psA", bufs=2, space="PSUM") as psA,
            tc.tile_pool(name="psB", bufs=2, space="PSUM") as psB,
            ExitStack() as _reps_stack,
        ):
            if reps > 1:
                _reps_stack.enter_context(tc.For_i(0, reps, 1))
            # ---------------- load constants ----------------
            onehot = pc.tile([MAXA, NPC], F32)
            nc.sync.dma_start(onehot[:], I_onehot[:])
            emb_t = pc.tile([MAXA, H], F32)
            nc.sync.dma_start(emb_t[:], I_emb[:])
            alw = []
            for k in range(3):
                t = pc.tile([H, H], F32, tag=f"alw{k}")
                nc.sync.dma_start(t[:], I_alw[k * H:(k + 1) * H, :])
                alw.append(t)
            alb = pc.tile([H, 1], F32)
            nc.sync.dma_start(alb[:], I_alb[:].unsqueeze(1))
            tta = pc.tile([128, GPC], F32)
            ttb = pc.tile([128, GPC], F32)
            nc.sync.dma_start(tta[:], I_tta[:])
            nc.sync.dma_start(ttb[:], I_ttb[:])
            ipt = pc.tile([9, GPC], F32)
            nc.sync.dma_start(ipt[:], I_ipt[:])
            cw = pc.tile([H, 3], F32)
            nc.sync.dma_start(cw[:], I_cw[:])
            lw = pc.tile([H, 9], F32)
            nc.sync.dma_start(lw[:], I_lw[:])

            w1a, w1b, w1c, wdr, w2r = [], [], [], [], []
            eb1, eb2, nw1a, nw1b, nb1, nw2, nb2 = [], [], [], [], [], [], []
            for l in range(L):
                t = pc.tile([H, H], F32, tag=f"w1a{l}")
                nc.sync.dma_start(t[:], I_ew1[l, 0:H, :])
                w1a.append(t)
                t = pc.tile([H, H], F32, tag=f"w1b{l}")
                nc.sync.dma_start(t[:], I_ew1[l, H:2 * H, :])
                w1b.append(t)
                t = pc.tile([9, H], F32, tag=f"w1c{l}")
                nc.sync.dma_start(t[:], I_ew1[l, 2 * H:2 * H + 9, :])
                w1c.append(t)
                # W1d duplicated at rows 0:60 and 64:124 so lhsT base
                # partition can match either dis2 band (0 or 64)
                wd = pw.tile([128, H], F32, tag="wd_stage")
                nc.sync.dma_start(wd[0:60, :], I_ew1[l, 2 * H + 9:, :])
                nc.sync.dma_start(wd[64:124, :], I_ew1[l, 2 * H + 9:, :])
                wdr_t = pc.tile([128, H], EDGE_DT, tag=f"wdr{l}")
                nc.vector.tensor_copy(wdr_t[0:60, :], wd[0:60, :])
                nc.vector.tensor_copy(wdr_t[64:124, :], wd[64:124, :])
                wdr.append(wdr_t)
                w2 = pw.tile([H, H], F32, tag="w2_stage")
                nc.sync.dma_start(w2[:], I_ew2[l])
                w2r_t = pc.tile([H, H], EDGE_DT, tag=f"w2r{l}")
                nc.vector.tensor_copy(w2r_t[:], w2[:])
                w2r.append(w2r_t)
                for lst, srcw, tag in ((eb1, I_eb1, "eb1"), (eb2, I_eb2, "eb2"),
                                       (nb1, I_nb1, "nb1"), (nb2, I_nb2, "nb2")):
                    t = pc.tile([H, 1], F32, tag=f"{tag}{l}")
                    nc.sync.dma_start(t[:], srcw[l].unsqueeze(1))
                    lst.append(t)
                t = pc.tile([H, H], F32, tag=f"nw1a{l}")
                nc.sync.dma_start(t[:], I_nw1[l, 0:H, :])
                nw1a.append(t)
                t = pc.tile([H, H], F32, tag=f"nw1b{l}")
                nc.sync.dma_start(t[:], I_nw1[l, H:2 * H, :])
                nw1b.append(t)
                t = pc.tile([H, H], F32, tag=f"nw2_{l}")
                nc.sync.dma_start(t[:], I_nw2[l])
                nw2.append(t)

            # ---------------- per-node sin/cos (once) ----------------
            xrep = p1.tile([R3F, NPB], F32)
            nc.sync.dma_start(xrep[:], I_xrep[:])
            freqc = p1.tile([R3F, 1], F32)
            nc.sync.dma_start(freqc[:], I_freq[:])

            v = p1.tile([R3F, NPB], F32, tag="trig_v")
            nc.vector.tensor_scalar_mul(v[:], xrep[:], freqc[:])  # v = f*x
            s4 = pc.tile([R3F, NPB], F32)
            c4 = pc.tile([R3F, NPB], F32)
            for out_t, offs in ((s4, 0.0), (c4, 0.25)):
                vv = pw.tile([R3F, NPB], F32, tag="trig_vv")
                if offs:
                    nc.vector.tensor_scalar_add(vv[:], v[:], offs)
                else:
                    nc.vector.tensor_copy(vv[:], v[:])
                r = pw.tile([R3F, NPB], F32, tag="trig_r")
                nc.vector.tensor_scalar_add(r[:], vv[:], MAGIC)
                nc.vector.tensor_scalar_add(r[:], r[:], -MAGIC)
                u = pw.tile([R3F, NPB], F32, tag="trig_u")
                nc.vector.tensor_tensor(u[:], vv[:], r[:], ALU.subtract)
                nc.vector.tensor_scalar_mul(u[:], u[:], TWOPI_SAFE)
                nc.scalar.activation(out_t[:], u[:], AF.Sin)

            # ---------------- edge sinusoids via angle addition -----------
            # chunk = one graph-pair across all 4 bands: [120, 1152]
            dis2 = pc.tile([128, CPB2], EDGE_DT)  # band b2 at rows 64*b2
            CH = 2 * NPG * NPG          # 1152
            for gp in range(GPB // 2):
                csl = slice(gp * 48, gp * 48 + 48)

                def jb(t):
                    a = t[:, csl].rearrange("p (g n) -> p g n", g=2)
                    return a.unsqueeze(2).broadcast_to([R3F, 2, NPG, NPG])

                def ib(t):
                    a = t[:, csl].rearrange("p (g n) -> p g n", g=2)
                    return a.unsqueeze(3).broadcast_to([R3F, 2, NPG, NPG])

                mv = lambda t: t[:].rearrange("p (g i j) -> p g i j", g=2, i=NPG)
                m1 = pt.tile([R3F, CH], F32, tag="m1")
                m2 = pt.tile([R3F, CH], F32, tag="m2")
                nc.vector.tensor_tensor(mv(m1), jb(s4), ib(c4), ALU.mult)
                nc.vector.tensor_tensor(mv(m2), jb(c4), ib(s4), ALU.mult)
                dsin = pt.tile([R3F, CH], EDGE_DT, tag="dsin")
                nc.vector.tensor_tensor(dsin[:], m1[:], m2[:], ALU.subtract)
                m3 = pt.tile([R3F, CH], F32, tag="m1")
                m4 = pt.tile([R3F, CH], F32, tag="m2")
                nc.vector.tensor_tensor(mv(m3), jb(c4), ib(c4), ALU.mult)
                nc.vector.tensor_tensor(mv(m4), jb(s4), ib(s4), ALU.mult)
                dcos = pt.tile([R3F, CH], EDGE_DT, tag="dcos")
                nc.vector.tensor_tensor(dcos[:], m3[:], m4[:], ALU.add)
                # repack into dis2
                for b4 in range(NBAND):
                    b2, hf = b4 // 2, b4 % 2
                    c0 = hf * CPB + gp * CH
                    nc.sync.dma_start(
                        dis2[64 * b2:64 * b2 + 30, c0:c0 + CH],
                        dsin[30 * b4:30 * b4 + 30, :])
                    nc.sync.dma_start(
                        dis2[64 * b2 + 30:64 * b2 + 60, c0:c0 + CH],
                        dcos[30 * b4:30 * b4 + 30, :])

            # ---------------- C for all layers [128, L*GPC] ----------------
            csb = pc.tile([H, L * GPC], F32)
            pC = psA.tile([128, 1024], F32, tag="psA")
            for l in range(L):
                nc.tensor.matmul(pC[:, l * GPC:(l + 1) * GPC], w1c[l][:], ipt[:],
                                 start=True, stop=True)
            nc.scalar.activation(csb[:], pC[:, 0:L * GPC], AF.Copy)

            def two_chunk(ps):   # [128, 2, 384] view of a [128,1024] psum tile
                return ps[:].rearrange("p (k x) -> p k x", k=2)[:, :, 0:384]

            def v2(t):           # [128, 2, 384] view of a [128,768] sbuf tile
                return t[:].rearrange("p (k x) -> p k x", k=2)

            # ---------------- node embedding + time conditioning ----------
            pE = psA.tile([128, 1024], F32, tag="psA")
            for k in range(2):
                nc.tensor.matmul(pE[:, k * 512:k * 512 + 384], emb_t[:],
                                 onehot[:, k * 384:(k + 1) * 384],
                                 start=True, stop=True)
            hemb = p1.tile([H, NPC], F32, tag="hemb")
            nc.scalar.activation(v2(hemb), two_chunk(pE), AF.Copy)

            pH = psB.tile([128, 1024], F32, tag="psB")
            for k in range(2):
                sl = pH[:, k * 512:k * 512 + 384]
                nc.tensor.matmul(sl, alw[0][:], hemb[:, k * 384:(k + 1) * 384],
                                 start=True, stop=False)
                gsl = slice(k * 16, (k + 1) * 16)
                rhs_a = tta[:, gsl].unsqueeze(2).broadcast_to([128, 16, NPG])
                nc.tensor.matmul(sl.rearrange("p (g n) -> p g n", g=16),
                                 alw[1][:], rhs_a, start=False, stop=False)
                rhs_b = ttb[:, gsl].unsqueeze(2).broadcast_to([128, 16, NPG])
                nc.tensor.matmul(sl.rearrange("p (g n) -> p g n", g=16),
                                 alw[2][:], rhs_b, start=False, stop=True)
            hT = pw.tile([H, NPC], F32, tag="hT")
            nc.scalar.activation(v2(hT), two_chunk(pH), AF.Identity, bias=alb[:])

            # ---------------- message-passing layers ----------------
            for l in range(L):
                # A' = W1a.T @ hT + C bcast ;  B = W1b.T @ hT   (fp32 mms)
                pA = psA.tile([128, 1024], F32, tag="psA")
                pB = psB.tile([128, 1024], F32, tag="psB")
                for k in range(2):
                    nc.tensor.matmul(pA[:, k * 512:k * 512 + 384], w1a[l][:],
                                     hT[:, k * 384:(k + 1) * 384],
                                     start=True, stop=True)
                    nc.tensor.matmul(pB[:, k * 512:k * 512 + 384], w1b[l][:],
                                     hT[:, k * 384:(k + 1) * 384],
                                     start=True, stop=True)
                aP = p1.tile([H, NPC], F32, tag="aP")
                cb = csb[:, l * GPC:(l + 1) * GPC]
                cb4 = (cb.rearrange("p (k g) -> p k g", k=2)
                       .unsqueeze(3).broadcast_to([H, 2, 16, NPG]))
                in0 = (pA[:].rearrange("p (k x) -> p k x", k=2)[:, :, 0:384]
                       .rearrange("p k (g n) -> p k g n", n=NPG))
                out0 = (aP[:].rearrange("p (k g n) -> p k g n", k=2, g=16))
                nc.vector.tensor_tensor(out0, in0, cb4, ALU.add)
                bP = p1.tile([H, NPC], F32, tag="bP")
                nc.scalar.activation(
                    bP[:].rearrange("p (k x) -> p k x", k=2),
                    pB[:].rearrange("p (k x) -> p k x", k=2)[:, :, 0:384],
                    AF.Copy)

                aggsum = p1.tile([H, NPC], F32, tag="aggsum")
                for bb in range(NBAND):       # 8-graph blocks
                    b2, hf = bb // 2, bb % 2
                    ef1 = pe1.tile([H, CPB], EDGE_DT, tag="ef1")
                    for gp in range(GPB // 2):   # graph pairs
                        tmp2 = pw.tile([H, 2 * NPG * NPG], F32, tag="tmp2")
                        for gh in range(2):
                            gl = gp * 2 + gh
                            g = bb * GPB + gl
                            col0 = hf * CPB + gl * NPG * NPG
                            pD = psA.tile([128, 1024], F32, tag="psA")
                            for hh in range(2):
                                nc.tensor.matmul(
                                    pD[:, hh * 512:hh * 512 + 288],
                                    wdr[l][64 * b2:64 * b2 + 60, :],
                                    dis2[64 * b2:64 * b2 + 60,
                                         col0 + hh * 288:col0 + (hh + 1) * 288],
                                    start=True, stop=True)
                            # pass1: tmp2 slice = psumD + A' bcast   (DVE)
                            a_sl = aP[:, g * NPG:(g + 1) * NPG]
                            a_v = (a_sl.rearrange("p (h i) -> p h i", h=2)
                                   .unsqueeze(3).broadcast_to([H, 2, 12, NPG]))
                            o_v = (tmp2[:, gh * 576:(gh + 1) * 576]
                                   .rearrange("p (h i j) -> p h i j", h=2, i=12))
                            i_v = (pD[:].rearrange("p (h x) -> p h x", h=2)
                                   [:, :, 0:288]
                                   .rearrange("p h (i j) -> p h i j", i=12))
                            nc.vector.tensor_tensor(o_v, i_v, a_v, ALU.add)
                        # pass2 on gpsimd: pre1 = tmp2 + B bcast
                        g0 = bb * GPB + gp * 2
                        b_sl = bP[:, g0 * NPG:(g0 + 2) * NPG]
                        b_v = (b_sl.rearrange("p (g j) -> p g j", g=2)
                               .unsqueeze(2).broadcast_to([H, 2, NPG, NPG]))
                        pre1 = pw.tile([H, 2 * NPG * NPG], F32, tag="pre1")
                        pv = lambda t: t[:].rearrange("p (g i j) -> p g i j",
                                                      g=2, i=NPG)
                        nc.gpsimd.tensor_tensor(pv(pre1), pv(tmp2), b_v, ALU.add)
                        nc.scalar.activation(
                            ef1[:, gp * 1152:(gp + 1) * 1152], pre1[:],
                            AF.Silu, bias=eb1[l][:])
                    # second edge matmul + silu2 over the block
                    ef2 = pe2.tile([H, CPB], EDGE_DT, tag="ef2")
                    for pp in range(5):          # 9 x 512-col chunks
                        p2 = psB.tile([128, 1024], F32, tag="psB")
                        nchunk = 2 if pp < 4 else 1
                        for cc in range(nchunk):
                            c0 = (pp * 2 + cc) * 512
                            nc.tensor.matmul(p2[:, cc * 512:(cc + 1) * 512],
                                             w2r[l][:], ef1[:, c0:c0 + 512],
                                             start=True, stop=True)
                        nc.scalar.activation(
                            ef2[:, pp * 1024:pp * 1024 + nchunk * 512],
                            p2[:, 0:nchunk * 512], AF.Silu, bias=eb2[l][:])
                    # aggregation over j; bf16 input gets the DVE 2x mode
                    nc.vector.tensor_reduce(
                        aggsum[:, bb * NPB:(bb + 1) * NPB],
                        ef2[:].rearrange("p (n j) -> p n j", j=NPG),
                        axis=mybir.AxisListType.X, op=ALU.add)
                aggT = p1.tile([H, NPC], F32, tag="aggT")
                nc.vector.tensor_scalar_mul(aggT[:], aggsum[:], 1.0 / NPG)

                # node MLP + residual
                pN1 = psA.tile([128, 1024], F32, tag="psA")
                for k in range(2):
                    sl = pN1[:, k * 512:k * 512 + 384]
                    nc.tensor.matmul(sl, nw1a[l][:],
                                     hT[:, k * 384:(k + 1) * 384],
                                     start=True, stop=False)
                    nc.tensor.matmul(sl, nw1b[l][:],
                                     aggT[:, k * 384:(k + 1) * 384],
                                     start=False, stop=True)
                upd1 = p1.tile([H, NPC], F32, tag="upd1")
                nc.scalar.activation(v2(upd1), two_chunk(pN1), AF.Silu,
                                     bias=nb1[l][:])
                pN2 = psB.tile([128, 1024], F32, tag="psB")
                for k in range(2):
                    nc.tensor.matmul(pN2[:, k * 512:k * 512 + 384], nw2[l][:],
                                     upd1[:, k * 384:(k + 1) * 384],
                                     start=True, stop=True)
                upd2 = p1.tile([H, NPC], F32, tag="upd2")
                nc.scalar.activation(v2(upd2), two_chunk(pN2), AF.Silu,
                                     bias=nb2[l][:])
                hT_new = pw.tile([H, NPC], F32, tag="hT")
                nc.vector.tensor_tensor(hT_new[:], hT[:], upd2[:], ALU.add)
                hT = hT_new

            # ---------------- outputs ----------------
            pF = psA.tile([128, 1024], F32, tag="psA")
            for k in range(2):
                nc.tensor.matmul(pF[0:3, k * 512:k * 512 + 384], cw[:],
                                 hT[:, k * 384:(k + 1) * 384],
                                 start=True, stop=True)
            coordT = p1.tile([3, NPC], F32, tag="coordT")
            nc.scalar.activation(
                coordT[:].rearrange("p (k x) -> p k x", k=2),
                pF[0:3].rearrange("p (k x) -> p k x", k=2)[:, :, 0:384],
                AF.Copy)
            nc.sync.dma_start(O_coord[:], coordT[:])

            gsum = p1.tile([H, GPC], F32, tag="gsum")
            nc.vector.tensor_reduce(
                gsum[:], hT[:].rearrange("p (g n) -> p g n", g=GPC),
                axis=mybir.AxisListType.X, op=ALU.add)
            gfeat = p1.tile([H, GPC], F32, tag="gfeat")
            nc.vector.tensor_scalar_mul(gfeat[:], gsum[:], 1.0 / NPG)
            pG = psB.tile([128, 1024], F32, tag="psB")
            nc.tensor.matmul(pG[0:9, 0:GPC], lw[:], gfeat[:],
                             start=True, stop=True)
            latp = p1.tile([9, GPC], F32, tag="latp")
            nc.scalar.activation(latp[:], pG[0:9, 0:GPC], AF.Copy)
            nc.sync.dma_start(O_lat[:], latp[:])

    nc.compile()
    return nc


def _prep_inputs(inputs):
    """Host-side sharding + layout prep. Index/layout work only."""
    t = np.asarray(inputs["t"], np.float32)
    frac = np.asarray(inputs["frac_coords"], np.float32)
    lf = np.asarray(inputs["lattice_feats"], np.float32)
    atom_types = np.asarray(inputs["atom_types"])
    node2graph = np.asarray(inputs["node2graph"])
    edge_index = np.asarray(inputs["edge_index"])

    # verify the fully-connected per-graph structure this kernel exploits
    assert node2graph.shape == (N,) and edge_index.shape == (2, E)
    exp_n2g = np.repeat(np.arange(G), NPG)
    assert np.array_equal(node2graph, exp_n2g), "unexpected node2graph"
    offs = (np.arange(G) * NPG)[:, None, None]
    ii = np.arange(NPG)
    exp_src = np.broadcast_to(offs + ii[:, None], (G, NPG, NPG)).reshape(-1)
    exp_dst = np.broadcast_to(offs + ii[None, :], (G, NPG, NPG)).reshape(-1)
    assert np.array_equal(edge_index[0], exp_src), "unexpected edge src"
    assert np.array_equal(edge_index[1], exp_dst), "unexpected edge dst"

    lat_ip = np.einsum("gij,gkj->gik", lf, lf).reshape(G, 9).astype(np.float32)

    freqcol = np.tile(np.arange(NFREQ, dtype=np.float32),
                      NBAND * 3).reshape(-1, 1)

    shared = {
        "emb_table": np.asarray(inputs["emb_table"], np.float32),
        "alw": np.asarray(inputs["atom_latent_w"], np.float32),
        "alb": np.asarray(inputs["atom_latent_b"], np.float32),
        "ew1": np.asarray(inputs["edge_w1"], np.float32),
        "eb1": np.asarray(inputs["edge_b1"], np.float32),
        "ew2": np.asarray(inputs["edge_w2"], np.float32),
        "eb2": np.asarray(inputs["edge_b2"], np.float32),
        "nw1": np.asarray(inputs["node_w1"], np.float32),
        "nb1": np.asarray(inputs["node_b1"], np.float32),
        "nw2": np.asarray(inputs["node_w2"], np.float32),
        "nb2": np.asarray(inputs["node_b2"], np.float32),
        "coord_w": np.asarray(inputs["coord_w"], np.float32),
        "lattice_w": np.asarray(inputs["lattice_w"], np.float32),
        "freqcol": freqcol,
    }

    in_maps = []
    for c in range(N_CORES):
        gsl = slice(c * GPC, (c + 1) * GPC)
        nsl = slice(c * NPC, (c + 1) * NPC)
        at = atom_types[nsl].astype(np.int64) - 1
        onehot = np.zeros((MAXA, NPC), np.float32)
        onehot[at, np.arange(NPC)] = 1.0
        xc = frac[nsl]                                 # [768,3]
        xb = xc.reshape(NBAND, NPB, 3).transpose(0, 2, 1)   # [4,3,192]
        xrep = np.repeat(xb[:, :, None, :], NFREQ, axis=2).reshape(120, NPB)
        tc_ = t[gsl]                                   # [32,256]
        in_maps.append(dict(shared,
                            onehot=onehot,
                            xrep=np.ascontiguousarray(xrep),
                            tta=np.ascontiguousarray(tc_[:, 0:128].T),
                            ttb=np.ascontiguousarray(tc_[:, 128:256].T),
                            ipt=np.ascontiguousarray(lat_ip[gsl].T)))
    return in_maps


def kernel(**inputs):
    if "nc" not in _CACHE:
        _CACHE["nc"] = _build()
    nc = _CACHE["nc"]
    in_maps = _prep_inputs(inputs)
    res = run_bass_kernel_spmd(nc, in_maps, list(range(N_CORES)))
    coord = np.concatenate([res.results[c]["coordT"].T
                            for c in range(N_CORES)], axis=0)
    lo = np.concatenate([res.results[c]["latpre"].T.reshape(GPC, 3, 3)
                         for c in range(N_CORES)], axis=0)
    lattices = np.asarray(inputs["lattices"], np.float32)
    lattice_out = np.einsum("bij,bjk->bik", lo, lattices).astype(np.float32)
    return lattice_out, coord.astype(np.float32)
